# revision 29
# baseline (speedup 1.0000x reference)
"""AFT-full kernel for Trainium2, SPMD across 8 NeuronCores.

Math (per batch b):
    q = in1 @ Wq.T + bq ; k = in1 @ Wk.T + bk ; v = in2 @ Wv.T + bv
    num = exp(position_biases) @ (exp(k) * v)      # [t, d]
    den = exp(position_biases) @ exp(k)            # [t, d]
    out = sigmoid(q) * num / den

Sharding: pure data parallel — core i computes batch i (BS == 8 == n_cores).
Weights / biases / position_biases are replicated to every core.

Per-core dataflow (matmuls in bf16, accumulation in f32 PSUM):
  - Startup in two waves. Wave A: W + in1 loads into DEDICATED f32 slots
    (all immediately issuable -> the DMA heaps drain every copy before
    any transpose becomes ready), DVE casts to bf16, then xbar-transpose
    burst A (W^T, x1^T). Wave B: in2 reusing the x slots, burst B (x2^T).
    Batching the transposes matters: every copy<->transpose xbar-mode
    transition serializes against in-flight DMAs (~10us on silicon).
  - Phase KV: k/v projections, ACT exp -> ek, DVE mul -> ekv.
  - Phase Q: q projections + sigmoid (single ACT LUT swap).
  - Phase T (pairs of t-tiles): position-bias panels PRE=2 pairs ahead;
    32 accumulating matmuls per tile (num/den share each stationary);
    DVE epilogue sigmoid(q) * num * approx(1/den); one store per pair.
  - position_biases panels: for symmetric pb (detected at runtime) the
    panel is loaded directly in [s on partitions] layout with a strided
    column DMA (1KB runs) + exp - no transposes at all. General pb uses
    contiguous row loads + exp + one batched xbar transpose per pair.
  - Transpose-source tiles are write-once regions or slot pools whose
    readers are compute engines; a slot-reuse WAR race against the xbar
    DMA's completion accounting was observed on silicon, as was output
    corruption from transposes with non-contiguous destinations, and
    wholesale corruption from >2048-column transposes. Keep transpose
    sources/destinations contiguous and <= [128, 16, 128].
"""

import sys

for _p in ("/opt/trn_rl_repo",):
    if _p not in sys.path:
        sys.path.insert(0, _p)

from contextlib import ExitStack

import numpy as np

import concourse.bass as bass
from concourse import bacc
import concourse.tile as tile
from concourse import mybir
from concourse.bass_utils import run_bass_kernel_spmd
from concourse.masks import make_identity

P = 128
N = 2048          # sequence length (n == s == t)
D = 512           # d_model
BS = 8            # batch size == number of cores
NT = N // P       # 16 row tiles
KT = D // P       # 4 contraction tiles for projections
XG = 4            # x row-tiles per transpose group
NG = NT // XG     # 4 groups
TG = 2            # t-tiles per T-phase pair
NP = NT // TG     # 8 pairs
PRE = 2           # pb pipeline depth (pairs ahead)
F32 = mybir.dt.float32
BF16 = mybir.dt.bfloat16

_NC_CACHE = {}


def build_nc(with_bias: bool, sym_pb: bool) -> bass.Bass:
    nc = bacc.Bacc()

    in1 = nc.declare_dram_parameter("inputs1", [N, D], F32, isOutput=False)
    in2 = nc.declare_dram_parameter("inputs2", [N, D], F32, isOutput=False)
    Wq = nc.declare_dram_parameter("Wq", [D, D], F32, isOutput=False)
    Wk = nc.declare_dram_parameter("Wk", [D, D], F32, isOutput=False)
    Wv = nc.declare_dram_parameter("Wv", [D, D], F32, isOutput=False)
    bq = nc.declare_dram_parameter("bq", [D], F32, isOutput=False)
    bk = nc.declare_dram_parameter("bk", [D], F32, isOutput=False)
    bv = nc.declare_dram_parameter("bv", [D], F32, isOutput=False)
    pb = nc.declare_dram_parameter("position_biases", [N, N], F32, isOutput=False)
    out = nc.declare_dram_parameter("out", [N, D], F32, isOutput=True)

    with ExitStack() as ctx:
        tc = ctx.enter_context(tile.TileContext(nc))

        persist = ctx.enter_context(tc.tile_pool(name="persist", bufs=1))
        # wT[p, w, o_t, i_t, f] == W_w[o_t*P + f, i_t*P + p]
        wT = persist.tile([P, 3, KT, KT, P], BF16)
        # x1T[p, g, a*KT + i_t, f] == in1[(g*XG + a)*P + f, i_t*P + p]
        x1T = persist.tile([P, NG, XG * KT, P], BF16)
        ek_sb = persist.tile([P, NT, D], BF16)     # exp(k), s on partitions
        ekv_sb = persist.tile([P, NT, D], BF16)    # exp(k) * v
        qsig_sb = persist.tile([P, NT, D], BF16)   # sigmoid(q)

        const = ctx.enter_context(tc.tile_pool(name="const", bufs=1))
        ident = const.tile([P, P], BF16)
        make_identity(nc, ident)

        ones_t = bias_bf = None
        if with_bias:
            ones_t = const.tile([1, P], BF16)
            nc.vector.memset(ones_t, 1.0)
            bias_bf = const.tile([1, 3, D], BF16)
            for w_idx, b in enumerate((bq, bk, bv)):
                nc.gpsimd.dma_start(out=bias_bf[:, w_idx, :], in_=b[:])

        # ================= startup + phase KV ===========================
        with tc.tile_pool(name="xw", bufs=1) as xw, \
                tc.tile_pool(name="xwstage", bufs=1) as xwstage:
            # bf16 transpose sources: write-once
            x1b = xw.tile([P, NG, XG, D], BF16)
            x2b = xw.tile([P, NG, XG, D], BF16)
            x2T = xw.tile([P, NG, XG * KT, P], BF16)
            wbf = xw.tile([P, 3, KT, D], BF16)
            # ---- loads + casts: f32 staging rotates through shallow
            # slot pools. The resulting load/cast readiness waves make the
            # scheduler interleave the transpose burst below with the
            # copy stream; each copy<->transpose xbar-mode transition
            # drains in-flight DMAs. That serialization costs ~40us of
            # startup but is ALSO what makes the transposes reliable on
            # silicon: with fully dedicated slots (clean burst, no
            # interleave) consumers of transposed data raced the xbar
            # completion (~1% corrupted output on a cold first run).
            # Keep this shape: proven scramble-clean on HW.
            for w_idx, W in enumerate((Wq, Wk, Wv)):
                for o_t in range(KT):
                    wf = xwstage.tile([P, D], F32, tag="wf", bufs=3)
                    eng = nc.scalar if w_idx == 2 else nc.sync
                    eng.dma_start(out=wf, in_=W[o_t * P:(o_t + 1) * P, :])
                    nc.vector.tensor_copy(out=wbf[:, w_idx, o_t, :], in_=wf)
            for g in range(NG):
                x1f = xwstage.tile([P, XG, D], F32, tag="x1f", bufs=3)
                nc.sync.dma_start(
                    out=x1f,
                    in_=in1[g * XG * P:(g + 1) * XG * P, :].rearrange(
                        "(a p) d -> p a d", p=P),
                )
                x2f = xwstage.tile([P, XG, D], F32, tag="x2f", bufs=2)
                nc.scalar.dma_start(
                    out=x2f,
                    in_=in2[g * XG * P:(g + 1) * XG * P, :].rearrange(
                        "(a p) d -> p a d", p=P),
                )
                nc.vector.tensor_copy(out=x1b[:, g, :, :], in_=x1f)
                nc.vector.tensor_copy(out=x2b[:, g, :, :], in_=x2f)

            # ---- PE transposes (TensorE + PSUM eviction). Slower than
            # the DMA xbar on paper, but the xbar's completion accounting
            # raced consumers on silicon in every schedule shape tried;
            # PE instruction completion is exact. PE is idle here anyway
            # and this warms the HAM clock gate before the matmul stream.
            with tc.tile_pool(name="tpsum", bufs=3, space="PSUM") as tpsum:
                def pe_t(dst, src):
                    # dst [P, KT, P] contiguous; src blocks [P, P]
                    tp = tpsum.tile([P, KT, P], BF16, tag="tp")
                    for i_t in range(KT):
                        nc.tensor.transpose(
                            tp[:, i_t, :], src[:, i_t * P:(i_t + 1) * P],
                            ident)
                    nc.vector.tensor_copy(out=dst, in_=tp)

                for w_idx in (1, 2, 0):       # k, v, q
                    for o_t in range(KT):
                        pe_t(wT[:, w_idx, o_t, :, :], wbf[:, w_idx, o_t, :])
                for g in range(NG):
                    for a in range(XG):
                        pe_t(x1T[:, g, a * KT:(a + 1) * KT, :],
                             x1b[:, g, a, :])
                        pe_t(x2T[:, g, a * KT:(a + 1) * KT, :],
                             x2b[:, g, a, :])

            def x1t_lhs(n_t, i_t):
                g, a = divmod(n_t, XG)
                return x1T[:, g, a * KT + i_t, :]

            # ---- phase KV: k/v projections, exp, ekv ----
            with tc.tile_pool(name="psum_kv", bufs=2, space="PSUM") as psum_kv:
                for n_t in range(NT):
                    g, a = divmod(n_t, XG)
                    psk = psum_kv.tile([P, D], F32, tag="psk")
                    psv = psum_kv.tile([P, D], F32, tag="psv")
                    for i_t in range(KT):
                        nc.tensor.matmul(
                            psk,
                            x1t_lhs(n_t, i_t),
                            wT[:, 1, :, i_t, :],
                            start=(i_t == 0),
                            stop=(i_t == KT - 1 and not with_bias),
                        )
                    for i_t in range(KT):
                        nc.tensor.matmul(
                            psv,
                            x2T[:, g, a * KT + i_t, :],
                            wT[:, 2, :, i_t, :],
                            start=(i_t == 0),
                            stop=(i_t == KT - 1 and not with_bias),
                        )
                    if with_bias:
                        nc.tensor.matmul(psk, ones_t, bias_bf[:, 1, :],
                                         start=False, stop=True)
                        nc.tensor.matmul(psv, ones_t, bias_bf[:, 2, :],
                                         start=False, stop=True)

                    nc.scalar.activation(
                        out=ek_sb[:, n_t, :], in_=psk,
                        func=mybir.ActivationFunctionType.Exp)
                    nc.vector.tensor_mul(
                        ekv_sb[:, n_t, :], ek_sb[:, n_t, :], psv)

        # ============ pb panel pipeline ==========
        pbpool = ctx.enter_context(tc.tile_pool(name="pbpool", bufs=PRE + 1))
        epi = ctx.enter_context(tc.tile_pool(name="epi", bufs=2))
        pbps = None
        if not sym_pb:
            pbps = ctx.enter_context(
                tc.tile_pool(name="pbps", bufs=2, space="PSUM"))
        panels = {}

        def pb_stage(j):
            if sym_pb:
                # Column-panel load: pbcol[p, s_t, c] = pb[s_t*P + p, j*TG*P + c]
                # (1KB contiguous runs). exp gives expB[s, t] which equals the
                # needed stationary expB[t, s] because pb is symmetric.
                pbcol = pbpool.tile([P, NT, TG * P], F32, tag="pbcol")
                nc.sync.dma_start(
                    out=pbcol,
                    in_=pb[:, j * TG * P:(j + 1) * TG * P].rearrange(
                        "(st p) t -> p st t", p=P),
                )
                panel = pbpool.tile([P, NT, TG * P], BF16, tag="panel")
                nc.scalar.activation(
                    out=panel, in_=pbcol,
                    func=mybir.ActivationFunctionType.Exp)
                # lhsT for (a, s_t) = panel[:, s_t, a*P:(a+1)*P]
                panels[j] = panel
            else:
                # General path: contiguous row load, exp, one batched xbar
                # transpose per pair.
                pbrow = pbpool.tile([P, TG, N], F32, tag="pbrow")
                nc.sync.dma_start(
                    out=pbrow,
                    in_=pb[j * TG * P:(j + 1) * TG * P, :].rearrange(
                        "(a p) s -> p a s", p=P),
                )
                pbexp = pbpool.tile([P, TG, N], BF16, tag="pbexp")
                nc.scalar.activation(
                    out=pbexp, in_=pbrow,
                    func=mybir.ActivationFunctionType.Exp)
                # panel[p, a*NT + s_t, f] == expB[(j*TG+a)*P + f, s_t*P + p]
                panel = pbpool.tile([P, TG * NT, P], BF16, tag="panel")
                for a in range(TG):
                    for sq in range(NT // KT):
                        tp2 = pbps.tile([P, KT, P], BF16, tag="tp2")
                        for u in range(KT):
                            s_t = sq * KT + u
                            nc.tensor.transpose(
                                tp2[:, u, :],
                                pbexp[:, a, s_t * P:(s_t + 1) * P], ident)
                        nc.vector.tensor_copy(
                            out=panel[:, a * NT + sq * KT:
                                      a * NT + (sq + 1) * KT, :],
                            in_=tp2)
                panels[j] = panel

        def panel_lhs(panel, a, s_t):
            if sym_pb:
                return panel[:, s_t, a * P:(a + 1) * P]
            return panel[:, a * NT + s_t, :]

        pb_stage(0)

        # ---- phase Q: q projections + sigmoid ----
        with tc.tile_pool(name="psum_q", bufs=3, space="PSUM") as psum_q:
            for n_t in range(NT):
                g, a = divmod(n_t, XG)
                psq = psum_q.tile([P, D], F32, tag="psq")
                for i_t in range(KT):
                    nc.tensor.matmul(
                        psq,
                        x1T[:, g, a * KT + i_t, :],
                        wT[:, 0, :, i_t, :],
                        start=(i_t == 0),
                        stop=(i_t == KT - 1 and not with_bias),
                    )
                if with_bias:
                    nc.tensor.matmul(psq, ones_t, bias_bf[:, 0, :],
                                     start=False, stop=True)
                nc.scalar.activation(
                    out=qsig_sb[:, n_t, :], in_=psq,
                    func=mybir.ActivationFunctionType.Sigmoid)

        pb_stage(1)

        psum_nd = ctx.enter_context(
            tc.tile_pool(name="psum_nd", bufs=2, space="PSUM"))

        # ---- phase T: num/den + epilogue per pair of t-tiles ----
        for j in range(NP):
            if j + PRE < NP:
                pb_stage(j + PRE)
            panel = panels.pop(j)

            pnum = psum_nd.tile([P, TG, D], F32, tag="pnum")
            pden = psum_nd.tile([P, TG, D], F32, tag="pden",
                                bufs=1 if not sym_pb else None)
            for a in range(TG):
                for s_t in range(NT):
                    lhsT = panel_lhs(panel, a, s_t)
                    nc.tensor.matmul(pnum[:, a, :], lhsT, ekv_sb[:, s_t, :],
                                     start=(s_t == 0), stop=(s_t == NT - 1))
                    nc.tensor.matmul(pden[:, a, :], lhsT, ek_sb[:, s_t, :],
                                     start=(s_t == 0), stop=(s_t == NT - 1))

            rec = epi.tile([P, TG, D], F32, tag="rec")
            nc.vector.reciprocal_approx_fast(out=rec, in_=pden)
            rat = epi.tile([P, TG, D], F32, tag="rat")
            nc.vector.tensor_mul(rat, rec, pnum)
            outt = epi.tile([P, TG, D], F32, tag="outt")
            nc.vector.tensor_mul(outt, rat, qsig_sb[:, j * TG:(j + 1) * TG, :])
            nc.sync.dma_start(
                out=out[j * TG * P:(j + 1) * TG * P, :].rearrange(
                    "(a p) d -> p a d", p=P),
                in_=outt,
            )

    nc.finalize()
    return nc


def _get_nc(with_bias: bool, sym_pb: bool) -> bass.Bass:
    key = (with_bias, sym_pb)
    if key not in _NC_CACHE:
        _NC_CACHE[key] = build_nc(with_bias, sym_pb)
    return _NC_CACHE[key]


def _make_in_maps(inputs: dict) -> list[dict]:
    in1 = np.ascontiguousarray(inputs["inputs1"], dtype=np.float32)
    in2 = np.ascontiguousarray(inputs["inputs2"], dtype=np.float32)
    shared = {
        k: np.ascontiguousarray(inputs[k], dtype=np.float32)
        for k in ("Wq", "Wk", "Wv", "bq", "bk", "bv", "position_biases")
    }
    return [
        {"inputs1": in1[c], "inputs2": in2[c], **shared}
        for c in range(BS)
    ]


def run(inputs: dict, trace: bool = False):
    """Returns (out [8,2048,512] f32, exec_time_ns or None)."""
    with_bias = any(
        np.any(np.asarray(inputs[b])) for b in ("bq", "bk", "bv"))
    pbv = np.asarray(inputs["position_biases"])
    sym_pb = bool(np.array_equal(pbv, pbv.T))
    nc = _get_nc(with_bias, sym_pb)
    in_maps = _make_in_maps(inputs)
    res = run_bass_kernel_spmd(
        nc, in_maps, core_ids=list(range(BS)), trace=trace)
    out = np.stack(
        [np.asarray(res.results[c]["out"]) for c in range(BS)], axis=0)
    return out.astype(np.float32), res.exec_time_ns


def kernel(**inputs) -> np.ndarray:
    out, _ = run(inputs, trace=False)
    return out
